# revision 53
# baseline (speedup 1.0000x reference)
"""GCN (2x GCNConv + classifier + log_softmax) on 8 Trainium2 NeuronCores.

Strategy (node sharding, per-core c owns rows [c*NPC, (c+1)*NPC)):
  h  = dinv * (x @ W1)      dense per-core rows from host-pre-transposed xT
                            (bf16); the src-side degree norm is folded into the
                            PSUM->fp8 cast. AllGather h (fp8e4).
  agg1 = Ahat @ h           dma_gather rows + one-hot matmul into PSUM per
                            128-dst block; the one-hot M is generated ON-CHIP
                            (DVE is_equal against an iota constant) from a tiny
                            dst-local table; self-loop terms are injected with
                            an identity-matmul of the core's own h block (no
                            gather, no virtual edges). flush = psum * dinv[dst]
                            (DVE) then h1T = Relu(+b1) on ScalarE, transposed
                            layout (features on partitions).
  z2 = dinv * (h1 @ W2)     dense from h1T; AllGather z2 (bf16)
  agg2 = Ahat @ z2          same SpMM -> h2T = Relu(psum*dinv[dst] + b2)
  logits = h2 @ Wc + bc     dense from h2T; batched log_softmax on free axis

Edge handling (host planner): edges are partitioned by dst-owner core, split
into lo/hi streams by src half (dma_gather int16 index limit), grouped by
128-dst block. Gather calls are per (block, half) (<=8 tiles each); pad slots
use trailing -1 indices which generate NO DMA descriptors; the per-core real
index count is loaded into a GpSimd register (reg_load) per call. Gather pool
buffers are memset once so stale pad lanes stay finite (M columns for pads are
zero, so they contribute nothing). The tile schedule is the max across cores so
one SPMD program serves every core.
"""
import sys
import numpy as np

sys.path.insert(0, '/opt/trn_rl_repo')

import ml_dtypes

BF16NP = ml_dtypes.bfloat16
FP8NP = ml_dtypes.float8_e4m3fn

USE_FP8 = True       # store/gather h in fp8e4 (layer 1); bf16 otherwise
DEBUG = False        # add intermediate-dump DRAM outputs
# Gather calls chain 8 full tiles from each src-half stream (GpSimd per-call
# fixed cost dominates fragmented per-cell calls). Pad slots gather row 0:
# skipping them per-core via num_idxs_reg works but forces per-cell call
# fragmentation, which costs more GpSimd issue time than the pads cost DMA.

N, E_EXPECT, IN, HID, MID, OUT = 50000, 800000, 512, 256, 128, 64
NC = 8
TILE = 128           # edges per matmul tile (contraction dim)
BLK = 128            # dst nodes per PSUM accumulation block
MAXCT = 8            # max tiles per dma_gather call (1024 rows; >1024 hard-
                     # crashes single-packet SWDGE on HW - verified)
MCHUNK = 16          # tiles per on-chip M generation chunk (DVE is_equal)
GBUFS = 20           # gather pool buffers


def _derived(n=None, nc=NC):
    if n is None:
        n = N
    npc = n // nc
    return dict(
        NPC=npc,
        NB128=(npc + 127) // 128,         # 128-row blocks (== SpMM dst blocks)
        HALF=n // 2,
    )


# ---------------------------------------------------------------- host planner

def plan(edge_index: np.ndarray, n=None, nc=NC):
    if n is None:
        n = N
    d = _derived(n, nc)
    npc, half = d['NPC'], d['HALF']
    nblk = d['NB128']

    src = edge_index[0].astype(np.int64)
    dst = edge_index[1].astype(np.int64)
    deg = np.ones(n, np.float64)
    np.add.at(deg, dst, 1.0)
    dinv = (1.0 / np.sqrt(deg)).astype(np.float32)

    core = dst // npc
    halfof = (src >= half).astype(np.int64)
    block = (dst % npc) // BLK
    dloc = (dst % npc) % BLK

    counts = np.zeros((nc, nblk, 2), np.int64)
    np.add.at(counts, (core, block, halfof), 1)
    maxcnt = counts.max(axis=0)               # [nblk, 2]

    # static schedule: tiles per (block, half) cell; gather calls chain 8
    # tiles at a time through each half's tile stream (block-major).
    sched = [[int(-(-maxcnt[b, h] // TILE)) for h in (0, 1)]
             for b in range(nblk)]
    TL = sum(s[0] for s in sched)
    TH = sum(s[1] for s in sched)
    NT = TL + TH
    # global (M/dl) tile index: block-major, lo tiles then hi tiles per block
    gbase = {}
    g = 0
    for b in range(nblk):
        for h in (0, 1):
            gbase[(b, h)] = g
            g += sched[b][h]

    def gather_layout(flat_idx):
        """[T*128] int -> [128, T*8] int16 in dma_gather layout, per 8-tile
        call chunks."""
        ntiles = len(flat_idx) // TILE
        cols = np.zeros((16, ntiles * 8), np.int16)
        pos = 0
        for c0 in range(0, ntiles, MAXCT):
            nt = min(MAXCT, ntiles - c0)
            nidx = nt * TILE
            cols[:, c0 * 8:c0 * 8 + nt * 8] = \
                flat_idx[pos:pos + nidx].reshape(-1, 16).T.astype(np.int16)
            pos += nidx
        return np.tile(cols, (8, 1))

    per_core = []
    for c in range(nc):
        m = core == c
        csrc, chalf = src[m], halfof[m]
        cblock, cdloc = block[m], dloc[m]
        dl_tab = np.full((128, NT), -1.0, np.float32)
        streams = {}
        for h in (0, 1):
            parts = []
            for b in range(nblk):
                tiles = sched[b][h]
                if tiles == 0:
                    continue
                mask = (cblock == b) & (chalf == h)
                sr = csrc[mask] - (half if h else 0)
                dls = cdloc[mask]
                ne = len(sr)
                cap = tiles * TILE
                assert ne <= cap
                ii = np.zeros(cap, np.int64)          # pads gather row 0
                ddl = np.full(cap, -1, np.int64)
                ii[:ne] = sr
                ddl[:ne] = dls
                parts.append(ii)
                g0 = gbase[(b, h)]
                for ti in range(tiles):
                    dl_tab[:, g0 + ti] = ddl[ti * TILE:(ti + 1) * TILE]
            streams[h] = gather_layout(np.concatenate(parts))
        per_core.append(dict(
            idx_lo=streams[0], idx_hi=streams[1],
            dl=dl_tab.astype(BF16NP)))

    return dict(sched=sched, NT=NT, TL=TL, TH=TH, per_core=per_core,
                dinv=dinv, d=d)


# ---------------------------------------------------------------- bass program

def build(nc_obj, sched, NT, TL, TH, n=None, ncores=NC):
    if n is None:
        n = N
    from concourse import bass, mybir, tile

    BF = mybir.dt.bfloat16
    F32 = mybir.dt.float32
    I16 = mybir.dt.int16
    FP8 = mybir.dt.float8e4 if USE_FP8 else BF

    d = _derived(n, ncores)
    NPC, NB128, HALF = d['NPC'], d['NB128'], d['HALF']
    NBLK = NB128
    NPAD = NB128 * 128
    NFULL = NPC // 128            # full 128-row blocks
    NREM = NPC - NFULL * 128      # rows in the final partial block

    b = nc_obj
    xT_d = b.declare_dram_parameter("xT", [IN, NPAD], BF, isOutput=False)
    W1_d = b.declare_dram_parameter("W1", [IN, HID], BF, isOutput=False)
    W2_d = b.declare_dram_parameter("W2", [HID, MID], BF, isOutput=False)
    Wc_d = b.declare_dram_parameter("Wc", [MID, OUT], BF, isOutput=False)
    b1_d = b.declare_dram_parameter("b1c", [128, HID // 128], F32, isOutput=False)
    b2_d = b.declare_dram_parameter("b2c", [128, MID // 128], F32, isOutput=False)
    bc_d = b.declare_dram_parameter("bcr", [128, OUT], F32, isOutput=False)
    il_d = b.declare_dram_parameter("idx_lo", [128, TL * 8], I16, isOutput=False)
    ih_d = b.declare_dram_parameter("idx_hi", [128, TH * 8], I16, isOutput=False)
    dl_d = b.declare_dram_parameter("dl", [128, NT], BF, isOutput=False)
    iota_d = b.declare_dram_parameter("iota", [128, MCHUNK * BLK], BF,
                                      isOutput=False)
    id8_d = b.declare_dram_parameter("id8", [128, 128], FP8, isOutput=False)
    id16_d = b.declare_dram_parameter("id16", [128, 128], BF, isOutput=False)
    dvo_d = b.declare_dram_parameter("dvo", [128, NB128], F32, isOutput=False)
    dvd_d = b.declare_dram_parameter("dvd", [128, NBLK * BLK], BF, isOutput=False)
    out_d = b.declare_dram_parameter("out", [NPC, OUT], F32, isOutput=True)
    if DEBUG:
        dbg_hb = b.declare_dram_parameter("dbg_hb", [128, NB128 * HID], FP8,
                                          isOutput=True)
        dbg_m = b.declare_dram_parameter("dbg_m", [128, MCHUNK * BLK], FP8,
                                         isOutput=True)
        dbg_g = b.declare_dram_parameter("dbg_g", [128, MAXCT * HID], FP8,
                                         isOutput=True)
        dbg_h1 = b.declare_dram_parameter("dbg_h1", [128, (HID // 128) * NBLK * BLK],
                                          BF, isOutput=True)
        dbg_hf = b.declare_dram_parameter("dbg_hf", [n, HID], FP8, isOutput=True)

    with tile.TileContext(b) as tc:
        # ---- resident SBUF tables
        W1_s, W1_s_free = tc.tile([128, IN // 128, HID], BF, name="W1s")
        W2_s, W2_s_free = tc.tile([128, HID // 128, MID], BF, name="W2s")
        Wc_s, Wc_s_free = tc.tile([128, OUT], BF, name="Wcs")
        b1_s, b1_s_free = tc.tile([128, HID // 128], F32, name="b1s")
        b2_s, b2_s_free = tc.tile([128, MID // 128], F32, name="b2s")
        bc_s, bc_s_free = tc.tile([128, OUT], F32, name="bcs")
        il_s, il_s_free = tc.tile([128, TL * 8], I16, name="ils")
        ih_s, ih_s_free = tc.tile([128, TH * 8], I16, name="ihs")
        dl_s, dl_s_free = tc.tile([128, NT], BF, name="dls")
        iota_s, iota_s_free = tc.tile([128, MCHUNK, BLK], BF, name="iotas")
        id8_s, id8_s_free = tc.tile([128, 128], FP8, name="id8s")
        id16_s, id16_s_free = tc.tile([128, 128], BF, name="id16s")
        dvo_s, dvo_s_free = tc.tile([128, NB128], F32, name="dvos")
        dvd_s, dvd_s_free = tc.tile([128, NBLK, BLK], BF, name="dvds")
        b.sync.dma_start(W1_s[:, :, :], W1_d.ap().rearrange("(k p) h -> p k h", p=128))
        b.sync.dma_start(W2_s[:, :, :], W2_d.ap().rearrange("(k p) h -> p k h", p=128))
        b.sync.dma_start(Wc_s[:, :], Wc_d.ap())
        b.sync.dma_start(b1_s[:, :], b1_d.ap())
        b.sync.dma_start(b2_s[:, :], b2_d.ap())
        b.sync.dma_start(bc_s[:, :], bc_d.ap())
        b.sync.dma_start(il_s[:, :], il_d.ap())
        b.sync.dma_start(ih_s[:, :], ih_d.ap())
        b.sync.dma_start(dl_s[:, :], dl_d.ap())
        b.sync.dma_start(iota_s[:, :, :],
                         iota_d.ap().rearrange("p (c k) -> p c k", k=BLK))
        b.sync.dma_start(id8_s[:, :], id8_d.ap())
        b.sync.dma_start(id16_s[:, :], id16_d.ap())
        b.sync.dma_start(dvo_s[:, :], dvo_d.ap())
        b.sync.dma_start(dvd_s[:, :, :],
                         dvd_d.ap().rearrange("p (c k) -> p c k", k=BLK))

        # DRAM staging (persist; declared first so SBUF frees stay LIFO).
        dshp = tc.alloc_tile_pool(name="dsh", bufs=1, space="DRAM")
        agin1 = dshp.tile([NPC, HID], FP8, name="agin1")
        h_full = dshp.tile([n, HID], FP8, name="h_full", addr_space="Shared")
        agin2 = dshp.tile([NPC, MID], BF, name="agin2")
        z2_full = dshp.tile([n, MID], BF, name="z2_full", addr_space="Shared")

        # ---- phase 1: h = dinv * (x @ W1), cast fp8, AllGather
        # xT is loaded in node-range chunks so the dense matmuls of chunk 0
        # overlap the load of chunk 1.
        hbf, hbf_free = tc.tile([128, NB128, HID], FP8, name="hbf")
        NXC = 4                               # node chunks
        CBLK = -(-NB128 // NXC)               # blocks per chunk
        with (tc.tile_pool(name="xtp", bufs=2) as xtp,
              tc.tile_pool(name="ps1p", bufs=2, space="PSUM") as ps1p):
            xT_r = xT_d.ap().rearrange("(k p) n -> p k n", p=128)
            for xc in range(NXC):
                b0, b1_ = xc * CBLK, min((xc + 1) * CBLK, NB128)
                xT_s = xtp.tile([128, IN // 128, CBLK * 128], BF, name="xTs")
                b.sync.dma_start(xT_s[:, :, :(b1_ - b0) * 128],
                                 xT_r[:, :, b0 * 128:b1_ * 128])
                for blk in range(b0, b1_):
                    ps1 = ps1p.tile([128, HID], F32, name="ps1")
                    o = (blk - b0) * 128
                    for k in range(IN // 128):
                        b.tensor.matmul(
                            ps1[:, :], lhsT=xT_s[:, k, o:o + 128],
                            rhs=W1_s[:, k, :], start=(k == 0),
                            stop=(k == IN // 128 - 1))
                    b.vector.tensor_scalar_mul(
                        out=hbf[:, blk, :], in0=ps1[:, :],
                        scalar1=dvo_s[:, blk:blk + 1])
                # stage this chunk's rows to DRAM while later chunks compute
                bf = min(b1_, NFULL)
                if bf > b0:
                    b.sync.dma_start(
                        agin1[b0 * 128:bf * 128, :].rearrange(
                            "(c p) f -> p c f", p=128),
                        hbf[:, b0:bf, :])
                if NREM and b1_ == NB128:
                    b.sync.dma_start(agin1[NFULL * 128:, :],
                                     hbf[:NREM, NFULL, :])
        b.gpsimd.collective_compute(
            "AllGather", mybir.AluOpType.bypass,
            replica_groups=[list(range(ncores))],
            ins=[agin1[:, :].opt()], outs=[h_full[:, :].opt()])

        # ---- SpMM machinery
        def spmm(feat_full, F, gdt, mdt, id_s, hown, hT_out, bias_s, layer,
                 on_block=None):
            """Aggregate feat_full (DRAM [n, F]) into
            hT_out [128, F//128, NBLK, BLK] = Relu(psum * dinv_dst + bias),
            features on partitions. hown is this core's own scaled-h tile
            ([128, NB128, F], node-major) used for the self-loop term."""
            nfc = F // 128
            streams = {0: feat_full[0:HALF, :], 1: feat_full[HALF:n, :]}
            gpool = tc.alloc_tile_pool(name=f"g{layer}", bufs=GBUFS)
            mpool = tc.alloc_tile_pool(name=f"m{layer}", bufs=4)
            pspool = tc.alloc_tile_pool(name=f"ps{layer}", bufs=4, space="PSUM")
            sbpool = tc.alloc_tile_pool(name=f"sb{layer}", bufs=4)

            dbg_done = [False, False]

            mstate = {}
            def ensure_m(mc):
                if mc in mstate:
                    return mstate[mc]
                t0 = mc * MCHUNK
                ntm = min(MCHUNK, NT - t0)
                mt = mpool.tile([128, MCHUNK, BLK], mdt, name=f"mt{layer}")
                b.vector.tensor_tensor(
                    out=mt[:, :ntm, :], in0=iota_s[:, :ntm, :],
                    in1=dl_s[:, t0:t0 + ntm].to_broadcast([128, ntm, BLK]),
                    op=mybir.AluOpType.is_equal)
                mstate.clear()
                mstate[mc] = mt
                if DEBUG and layer == 1 and not dbg_done[0]:
                    dbg_done[0] = True
                    b.sync.dma_start(
                        dbg_m.ap().rearrange("p (c k) -> p c k", k=BLK),
                        mt[:, :, :])
                return mt

            idx_tabs = {0: il_s, 1: ih_s}
            ntiles_h = {0: TL, 1: TH}
            gmap = {0: {}, 1: {}}
            gcnt = {0: 0, 1: 0}

            def ensure_gather(s, th):
                while th >= gcnt[s] * MAXCT:
                    c0 = gcnt[s] * MAXCT
                    nt = min(MAXCT, ntiles_h[s] - c0)
                    gt = gpool.tile([128, MAXCT, F], gdt, name=f"gt{layer}{s}")
                    b.gpsimd.dma_gather(
                        out_ap=gt[:, :nt, :],
                        in_ap=streams[s],
                        idxs_ap=idx_tabs[s][:, c0 * 8:(c0 + nt) * 8],
                        num_idxs=nt * TILE,
                        num_idxs_reg=nt * TILE,
                        elem_size=F,
                        single_packet=True,
                        queue_num=(s * 2 + gcnt[s] % 2),
                    )
                    gmap[s][gcnt[s]] = gt
                    gcnt[s] += 1

            g = 0
            cons = {0: 0, 1: 0}
            for blk in range(NBLK):
                tot = sched[blk][0] + sched[blk][1]
                ps = [pspool.tile([128, BLK], F32, name=f"ps{layer}_{fc}")
                      for fc in range(nfc)]
                for fc in range(nfc):
                    b.tensor.matmul(
                        ps[fc][:, :], lhsT=hown[:, blk, fc * 128:(fc + 1) * 128],
                        rhs=id_s[:, :], start=True, stop=(tot == 0))
                ti_blk = 0
                for h in (0, 1):
                    for _ in range(sched[blk][h]):
                        th = cons[h]
                        ensure_gather(h, th)
                        gt = gmap[h][th // MAXCT]
                        gc = th % MAXCT
                        mt = ensure_m(g // MCHUNK)
                        mo = g % MCHUNK
                        last = (ti_blk == tot - 1)
                        for fc in range(nfc):
                            b.tensor.matmul(
                                ps[fc][:, :],
                                lhsT=gt[:, gc, fc * 128:(fc + 1) * 128],
                                rhs=mt[:, mo, :],
                                start=False, stop=last)
                        cons[h] += 1
                        g += 1
                        ti_blk += 1
                for fc in range(nfc):
                    tmp = sbpool.tile([128, BLK], BF, name=f"tp{layer}_{fc}")
                    b.vector.tensor_tensor(
                        out=tmp[:, :], in0=ps[fc][:, :], in1=dvd_s[:, blk, :],
                        op=mybir.AluOpType.mult)
                    b.scalar.activation(
                        out=hT_out[:, fc, blk, :], in_=tmp[:, :],
                        func=mybir.ActivationFunctionType.Relu,
                        bias=bias_s[:, fc:fc + 1])
                if on_block is not None:
                    on_block(blk)
            assert g == NT and cons[0] == TL and cons[1] == TH
            for p in (sbpool, pspool, mpool, gpool):
                p.release()

        # ---- layer 1 SpMM -> h1T (hbf stays alive for the self term)
        z2bf, z2bf_free = tc.tile([128, NB128, MID], BF, name="z2bf")
        h1T, h1T_free = tc.tile([128, HID // 128, NBLK, BLK], BF, name="h1T")
        if DEBUG:
            b.sync.dma_start(dbg_hb.ap().rearrange("p (c k) -> p c k", k=HID),
                             hbf[:, :, :])
            b.sync.dma_start(dbg_hf.ap(), h_full[:, :])
        spmm(h_full, HID, FP8, FP8, id8_s, hbf, h1T, b1_s, layer=1)
        if DEBUG:
            b.sync.dma_start(
                dbg_h1.ap().rearrange("p (a c k) -> p a c k", c=NBLK, k=BLK),
                h1T[:, :, :, :])

        # ---- z2 = dinv * (h1 @ W2), AllGather (staged to DRAM per chunk)
        with tc.tile_pool(name="ps2p", bufs=2, space="PSUM") as ps2p:
            for xc in range(NXC):
                b0, b1_ = xc * CBLK, min((xc + 1) * CBLK, NB128)
                for blk in range(b0, b1_):
                    psz = ps2p.tile([128, MID], F32, name="psz")
                    for fc in range(HID // 128):
                        b.tensor.matmul(
                            psz[:, :],
                            lhsT=h1T[:, fc, blk, :],
                            rhs=W2_s[:, fc, :],
                            start=(fc == 0), stop=(fc == HID // 128 - 1))
                    b.vector.tensor_scalar_mul(
                        out=z2bf[:, blk, :], in0=psz[:, :],
                        scalar1=dvo_s[:, blk:blk + 1])
                bf = min(b1_, NFULL)
                if bf > b0:
                    b.sync.dma_start(
                        agin2[b0 * 128:bf * 128, :].rearrange(
                            "(c p) f -> p c f", p=128),
                        z2bf[:, b0:bf, :])
                if NREM and b1_ == NB128:
                    b.sync.dma_start(agin2[NFULL * 128:, :],
                                     z2bf[:NREM, NFULL, :])
        h1T_free()
        b.gpsimd.collective_compute(
            "AllGather", mybir.AluOpType.bypass,
            replica_groups=[list(range(ncores))],
            ins=[agin2[:, :].opt()], outs=[z2_full[:, :].opt()])

        # ---- layer 2 SpMM -> h2T (z2bf stays alive for the self term)
        h2T, h2T_free = tc.tile([128, MID // 128, NBLK, BLK], BF, name="h2T")
        spmm(z2_full, MID, BF, BF, id16_s, z2bf, h2T, b2_s, layer=2)

        # ---- logits + log_softmax
        ls_s, ls_s_free = tc.tile([128, NB128, OUT], F32, name="lss")
        ex_s, ex_s_free = tc.tile([128, NB128, OUT], F32, name="exs")
        mx_s, mx_s_free = tc.tile([128, NB128], F32, name="mxs")
        sm_s, sm_s_free = tc.tile([128, NB128], F32, name="sms")
        lg_s, lg_s_free = tc.tile([128, NB128], F32, name="lgs")
        with tc.tile_pool(name="ps3p", bufs=2, space="PSUM") as ps3p:
            for blk in range(NB128):
                psl = ps3p.tile([128, OUT], F32, name="psl")
                b.tensor.matmul(
                    psl[:, :],
                    lhsT=h2T[:, 0, blk, :],
                    rhs=Wc_s[:, :], start=True, stop=True)
                b.vector.tensor_tensor(out=ls_s[:, blk, :], in0=psl[:, :],
                                       in1=bc_s[:, :], op=mybir.AluOpType.add)
        b.vector.reduce_max(out=mx_s[:, :], in_=ls_s[:, :, :],
                            axis=mybir.AxisListType.X)
        b.vector.tensor_tensor(out=ls_s[:, :, :], in0=ls_s[:, :, :],
                               in1=mx_s[:, :].to_broadcast([128, NB128, OUT]),
                               op=mybir.AluOpType.subtract)
        b.scalar.activation(out=ex_s[:, :, :], in_=ls_s[:, :, :],
                            func=mybir.ActivationFunctionType.Exp)
        b.vector.reduce_sum(out=sm_s[:, :], in_=ex_s[:, :, :],
                            axis=mybir.AxisListType.X)
        b.scalar.activation(out=lg_s[:, :], in_=sm_s[:, :],
                            func=mybir.ActivationFunctionType.Ln)
        b.vector.tensor_tensor(out=ex_s[:, :, :], in0=ls_s[:, :, :],
                               in1=lg_s[:, :].to_broadcast([128, NB128, OUT]),
                               op=mybir.AluOpType.subtract)
        b.sync.dma_start(out_d.ap()[:NFULL * 128, :].rearrange(
            "(c p) f -> p c f", p=128), ex_s[:, :NFULL, :])
        if NREM:
            b.sync.dma_start(out_d.ap()[NFULL * 128:, :], ex_s[:NREM, NFULL, :])

        # release remaining pools in LIFO order (leftover TilePoolBoundary
        # pseudo-ops crash walrus)
        lg_s_free(); sm_s_free(); mx_s_free(); ex_s_free(); ls_s_free()
        h2T_free()
        z2bf_free()
        hbf_free()
        dshp.release()
        dvd_s_free(); dvo_s_free()
        id16_s_free(); id8_s_free(); iota_s_free()
        dl_s_free(); ih_s_free(); il_s_free()
        bc_s_free(); b2_s_free(); b1_s_free()
        Wc_s_free(); W2_s_free(); W1_s_free()

    return b


# ---------------------------------------------------------------- entry point

def make_in_maps(inputs, p, n=None, ncores=NC):
    if n is None:
        n = N
    d = p['d']
    NPC, NB128 = d['NPC'], d['NB128']
    NPAD = NB128 * 128
    NBLK = NB128
    x = np.asarray(inputs['x'], np.float32)
    W1 = np.asarray(inputs['W1'], np.float32).astype(BF16NP)
    W2 = np.asarray(inputs['W2'], np.float32).astype(BF16NP)
    Wc = np.asarray(inputs['Wc'], np.float32).astype(BF16NP)
    b1 = np.asarray(inputs['b1'], np.float32)
    b2 = np.asarray(inputs['b2'], np.float32)
    bc = np.asarray(inputs['bc'], np.float32)
    dinv = p['dinv']

    b1c = b1.reshape(HID // 128, 128).T.copy()
    b2c = b2.reshape(MID // 128, 128).T.copy()
    bcr = np.tile(bc[None, :], (128, 1))
    iota = np.tile(np.arange(BLK, dtype=np.float32)[None, :],
                   (128, MCHUNK)).astype(BF16NP)
    id8 = np.eye(128, dtype=np.float32).astype(FP8NP if USE_FP8 else BF16NP)
    id16 = np.eye(128, dtype=np.float32).astype(BF16NP)

    in_maps = []
    for c in range(ncores):
        rows = slice(c * NPC, (c + 1) * NPC)
        xT = np.zeros((IN, NPAD), BF16NP)
        xT[:, :NPC] = x[rows].T.astype(BF16NP)
        dv = dinv[rows]
        tmpv = np.zeros(NB128 * 128, np.float32)
        tmpv[:NPC] = dv
        dvo = tmpv.reshape(NB128, 128).T.copy()      # dinv by (row%128, row//128)
        dvd = np.zeros((128, NBLK * BLK), BF16NP)    # dinv[dst] replicated /parts
        dvd[:, :NPC] = np.tile(dv.astype(BF16NP)[None, :], (128, 1))
        pc = p['per_core'][c]
        in_maps.append(dict(
            xT=xT, W1=W1, W2=W2, Wc=Wc, b1c=b1c, b2c=b2c, bcr=bcr,
            idx_lo=pc['idx_lo'], idx_hi=pc['idx_hi'], dl=pc['dl'],
            iota=iota, id8=id8, id16=id16, dvo=dvo, dvd=dvd))
    return in_maps


def kernel_with_results(inputs, trace=False, **kw):
    from concourse import bacc
    from concourse import bass_utils

    edge_index = np.asarray(inputs['edge_index'])
    p = plan(edge_index, n=N)
    nc_obj = bacc.Bacc("TRN2", target_bir_lowering=False, debug=False,
                       num_devices=NC, num_swdge_queues=4)
    build(nc_obj, p['sched'], p['NT'], p['TL'], p['TH'], n=N)
    nc_obj.compile()
    in_maps = make_in_maps(inputs, p, n=N)
    res = bass_utils.run_bass_kernel_spmd(nc_obj, in_maps,
                                          core_ids=list(range(NC)),
                                          trace=trace, **kw)
    out = np.concatenate([np.asarray(res.results[c]['out']) for c in range(NC)],
                         axis=0)
    return out.astype(np.float32), res


def kernel(**inputs) -> np.ndarray:
    return kernel_with_results(inputs)[0]


if __name__ == '__main__':
    import reference
    inputs = {k: np.asarray(v) for k, v in reference.setup_inputs().items()}
    out = kernel(**inputs)
    print('kernel out', out.shape, out.dtype)


# revision 54
# speedup vs baseline: 1.1553x; 1.1553x over previous
"""GCN (2x GCNConv + classifier + log_softmax) on 8 Trainium2 NeuronCores.

Strategy (node sharding, per-core c owns rows [c*NPC, (c+1)*NPC)):
  h  = dinv * (x @ W1)      dense per-core rows from host-pre-transposed xT
                            (bf16); the src-side degree norm is folded into the
                            PSUM->fp8 cast. AllGather h (fp8e4).
  agg1 = Ahat @ h           dma_gather rows + one-hot matmul into PSUM per
                            128-dst block; the one-hot M is generated ON-CHIP
                            (DVE is_equal against an iota constant) from a tiny
                            dst-local table; self-loop terms are injected with
                            an identity-matmul of the core's own h block (no
                            gather, no virtual edges). flush = psum * dinv[dst]
                            (DVE) then h1T = Relu(+b1) on ScalarE, transposed
                            layout (features on partitions).
  z2 = dinv * (h1 @ W2)     dense from h1T; AllGather z2 (bf16)
  agg2 = Ahat @ z2          same SpMM -> h2T = Relu(psum*dinv[dst] + b2)
  logits = h2 @ Wc + bc     dense from h2T; batched log_softmax on free axis

Edge handling (host planner): edges are partitioned by dst-owner core, split
into lo/hi streams by src half (dma_gather int16 index limit), grouped by
128-dst block. Gather calls are per (block, half) (<=8 tiles each); pad slots
use trailing -1 indices which generate NO DMA descriptors; the per-core real
index count is loaded into a GpSimd register (reg_load) per call. Gather pool
buffers are memset once so stale pad lanes stay finite (M columns for pads are
zero, so they contribute nothing). The tile schedule is the max across cores so
one SPMD program serves every core.
"""
import sys
import numpy as np

sys.path.insert(0, '/opt/trn_rl_repo')

import ml_dtypes

BF16NP = ml_dtypes.bfloat16
FP8NP = ml_dtypes.float8_e4m3fn

USE_FP8 = True       # store/gather h in fp8e4 (layer 1); bf16 otherwise
DEBUG = False        # add intermediate-dump DRAM outputs
# Gather calls chain 8 full tiles from each src-half stream (GpSimd per-call
# fixed cost dominates fragmented per-cell calls). Pad slots gather row 0:
# skipping them per-core via num_idxs_reg works but forces per-cell call
# fragmentation, which costs more GpSimd issue time than the pads cost DMA.

N, E_EXPECT, IN, HID, MID, OUT = 50000, 800000, 512, 256, 128, 64
NC = 8
TILE = 128           # edges per matmul tile (contraction dim)
BLK = 128            # dst nodes per PSUM accumulation block
MAXCT = 8            # max tiles per dma_gather call (1024 rows; >1024 hard-
                     # crashes single-packet SWDGE on HW - verified)
MCHUNK = 8           # tiles per on-chip M generation chunk (DVE is_equal)
GBUFS = 16           # gather pool buffers


def _derived(n=None, nc=NC):
    if n is None:
        n = N
    npc = n // nc
    return dict(
        NPC=npc,
        NB128=(npc + 127) // 128,         # 128-row blocks (== SpMM dst blocks)
        HALF=n // 2,
    )


# ---------------------------------------------------------------- host planner

def plan(edge_index: np.ndarray, n=None, nc=NC):
    if n is None:
        n = N
    d = _derived(n, nc)
    npc, half = d['NPC'], d['HALF']
    nblk = d['NB128']

    src = edge_index[0].astype(np.int64)
    dst = edge_index[1].astype(np.int64)
    deg = np.ones(n, np.float64)
    np.add.at(deg, dst, 1.0)
    dinv = (1.0 / np.sqrt(deg)).astype(np.float32)

    core = dst // npc
    halfof = (src >= half).astype(np.int64)
    block = (dst % npc) // BLK
    dloc = (dst % npc) % BLK

    counts = np.zeros((nc, nblk, 2), np.int64)
    np.add.at(counts, (core, block, halfof), 1)
    maxcnt = counts.max(axis=0)               # [nblk, 2]

    # static schedule: tiles per (block, half) cell; gather calls chain 8
    # tiles at a time through each half's tile stream (block-major).
    sched = [[int(-(-maxcnt[b, h] // TILE)) for h in (0, 1)]
             for b in range(nblk)]
    TL = sum(s[0] for s in sched)
    TH = sum(s[1] for s in sched)
    NT = TL + TH
    # global (M/dl) tile index: block-major, lo tiles then hi tiles per block
    gbase = {}
    g = 0
    for b in range(nblk):
        for h in (0, 1):
            gbase[(b, h)] = g
            g += sched[b][h]

    def gather_layout(flat_idx):
        """[T*128] int -> [128, T*8] int16 in dma_gather layout, per 8-tile
        call chunks."""
        ntiles = len(flat_idx) // TILE
        cols = np.zeros((16, ntiles * 8), np.int16)
        pos = 0
        for c0 in range(0, ntiles, MAXCT):
            nt = min(MAXCT, ntiles - c0)
            nidx = nt * TILE
            cols[:, c0 * 8:c0 * 8 + nt * 8] = \
                flat_idx[pos:pos + nidx].reshape(-1, 16).T.astype(np.int16)
            pos += nidx
        return np.tile(cols, (8, 1))

    per_core = []
    for c in range(nc):
        m = core == c
        csrc, chalf = src[m], halfof[m]
        cblock, cdloc = block[m], dloc[m]
        dl_tab = np.full((128, NT), -1.0, np.float32)
        streams = {}
        for h in (0, 1):
            parts = []
            for b in range(nblk):
                tiles = sched[b][h]
                if tiles == 0:
                    continue
                mask = (cblock == b) & (chalf == h)
                sr = csrc[mask] - (half if h else 0)
                dls = cdloc[mask]
                ne = len(sr)
                cap = tiles * TILE
                assert ne <= cap
                ii = np.zeros(cap, np.int64)          # pads gather row 0
                ddl = np.full(cap, -1, np.int64)
                ii[:ne] = sr
                ddl[:ne] = dls
                parts.append(ii)
                g0 = gbase[(b, h)]
                for ti in range(tiles):
                    dl_tab[:, g0 + ti] = ddl[ti * TILE:(ti + 1) * TILE]
            streams[h] = gather_layout(np.concatenate(parts))
        per_core.append(dict(
            idx_lo=streams[0], idx_hi=streams[1],
            dl=dl_tab.astype(BF16NP)))

    return dict(sched=sched, NT=NT, TL=TL, TH=TH, per_core=per_core,
                dinv=dinv, d=d)


# ---------------------------------------------------------------- bass program

def build(nc_obj, sched, NT, TL, TH, n=None, ncores=NC):
    if n is None:
        n = N
    from concourse import bass, mybir, tile

    BF = mybir.dt.bfloat16
    F32 = mybir.dt.float32
    I16 = mybir.dt.int16
    FP8 = mybir.dt.float8e4 if USE_FP8 else BF

    d = _derived(n, ncores)
    NPC, NB128, HALF = d['NPC'], d['NB128'], d['HALF']
    NBLK = NB128
    NPAD = NB128 * 128
    NFULL = NPC // 128            # full 128-row blocks
    NREM = NPC - NFULL * 128      # rows in the final partial block

    b = nc_obj
    xT_d = b.declare_dram_parameter("xT", [IN, NPAD], BF, isOutput=False)
    W1_d = b.declare_dram_parameter("W1", [IN, HID], BF, isOutput=False)
    W2_d = b.declare_dram_parameter("W2", [HID, MID], BF, isOutput=False)
    Wc_d = b.declare_dram_parameter("Wc", [MID, OUT], BF, isOutput=False)
    b1_d = b.declare_dram_parameter("b1c", [128, HID // 128], F32, isOutput=False)
    b2_d = b.declare_dram_parameter("b2c", [128, MID // 128], F32, isOutput=False)
    bc_d = b.declare_dram_parameter("bcr", [128, OUT], F32, isOutput=False)
    il_d = b.declare_dram_parameter("idx_lo", [128, TL * 8], I16, isOutput=False)
    ih_d = b.declare_dram_parameter("idx_hi", [128, TH * 8], I16, isOutput=False)
    dl_d = b.declare_dram_parameter("dl", [128, NT], BF, isOutput=False)
    iota_d = b.declare_dram_parameter("iota", [128, MCHUNK * BLK], BF,
                                      isOutput=False)
    id8_d = b.declare_dram_parameter("id8", [128, 128], FP8, isOutput=False)
    id16_d = b.declare_dram_parameter("id16", [128, 128], BF, isOutput=False)
    dvo_d = b.declare_dram_parameter("dvo", [128, NB128], F32, isOutput=False)
    dvd_d = b.declare_dram_parameter("dvd", [128, NBLK * BLK], BF, isOutput=False)
    out_d = b.declare_dram_parameter("out", [NPC, OUT], F32, isOutput=True)
    if DEBUG:
        dbg_hb = b.declare_dram_parameter("dbg_hb", [128, NB128 * HID], FP8,
                                          isOutput=True)
        dbg_m = b.declare_dram_parameter("dbg_m", [128, MCHUNK * BLK], FP8,
                                         isOutput=True)
        dbg_g = b.declare_dram_parameter("dbg_g", [128, MAXCT * HID], FP8,
                                         isOutput=True)
        dbg_h1 = b.declare_dram_parameter("dbg_h1", [128, (HID // 128) * NBLK * BLK],
                                          BF, isOutput=True)
        dbg_hf = b.declare_dram_parameter("dbg_hf", [n, HID], FP8, isOutput=True)

    with tile.TileContext(b) as tc:
        # ---- resident SBUF tables
        W1_s, W1_s_free = tc.tile([128, IN // 128, HID], BF, name="W1s")
        W2_s, W2_s_free = tc.tile([128, HID // 128, MID], BF, name="W2s")
        Wc_s, Wc_s_free = tc.tile([128, OUT], BF, name="Wcs")
        b1_s, b1_s_free = tc.tile([128, HID // 128], F32, name="b1s")
        b2_s, b2_s_free = tc.tile([128, MID // 128], F32, name="b2s")
        bc_s, bc_s_free = tc.tile([128, OUT], F32, name="bcs")
        il_s, il_s_free = tc.tile([128, TL * 8], I16, name="ils")
        ih_s, ih_s_free = tc.tile([128, TH * 8], I16, name="ihs")
        dl_s, dl_s_free = tc.tile([128, NT], BF, name="dls")
        iota_s, iota_s_free = tc.tile([128, MCHUNK, BLK], BF, name="iotas")
        id8_s, id8_s_free = tc.tile([128, 128], FP8, name="id8s")
        id16_s, id16_s_free = tc.tile([128, 128], BF, name="id16s")
        dvo_s, dvo_s_free = tc.tile([128, NB128], F32, name="dvos")
        dvd_s, dvd_s_free = tc.tile([128, NBLK, BLK], BF, name="dvds")
        b.sync.dma_start(W1_s[:, :, :], W1_d.ap().rearrange("(k p) h -> p k h", p=128))
        b.sync.dma_start(W2_s[:, :, :], W2_d.ap().rearrange("(k p) h -> p k h", p=128))
        b.sync.dma_start(Wc_s[:, :], Wc_d.ap())
        b.sync.dma_start(b1_s[:, :], b1_d.ap())
        b.sync.dma_start(b2_s[:, :], b2_d.ap())
        b.sync.dma_start(bc_s[:, :], bc_d.ap())
        b.sync.dma_start(il_s[:, :], il_d.ap())
        b.sync.dma_start(ih_s[:, :], ih_d.ap())
        b.sync.dma_start(dl_s[:, :], dl_d.ap())
        b.sync.dma_start(iota_s[:, :, :],
                         iota_d.ap().rearrange("p (c k) -> p c k", k=BLK))
        b.sync.dma_start(id8_s[:, :], id8_d.ap())
        b.sync.dma_start(id16_s[:, :], id16_d.ap())
        b.sync.dma_start(dvo_s[:, :], dvo_d.ap())
        b.sync.dma_start(dvd_s[:, :, :],
                         dvd_d.ap().rearrange("p (c k) -> p c k", k=BLK))

        # DRAM staging (persist; declared first so SBUF frees stay LIFO).
        dshp = tc.alloc_tile_pool(name="dsh", bufs=1, space="DRAM")
        agin1 = dshp.tile([NPC, HID], FP8, name="agin1")
        h_full = dshp.tile([n, HID], FP8, name="h_full", addr_space="Shared")
        agin2 = dshp.tile([NPC, MID], BF, name="agin2")
        z2_full = dshp.tile([n, MID], BF, name="z2_full", addr_space="Shared")

        # ---- phase 1: h = dinv * (x @ W1), cast fp8, AllGather
        # xT is loaded in node-range chunks so the dense matmuls of chunk 0
        # overlap the load of chunk 1.
        hbf, hbf_free = tc.tile([128, NB128, HID], FP8, name="hbf")
        NXC = 4                               # node chunks
        CBLK = -(-NB128 // NXC)               # blocks per chunk
        with (tc.tile_pool(name="xtp", bufs=2) as xtp,
              tc.tile_pool(name="ps1p", bufs=2, space="PSUM") as ps1p):
            xT_r = xT_d.ap().rearrange("(k p) n -> p k n", p=128)
            for xc in range(NXC):
                b0, b1_ = xc * CBLK, min((xc + 1) * CBLK, NB128)
                xT_s = xtp.tile([128, IN // 128, CBLK * 128], BF, name="xTs")
                b.sync.dma_start(xT_s[:, :, :(b1_ - b0) * 128],
                                 xT_r[:, :, b0 * 128:b1_ * 128])
                for blk in range(b0, b1_):
                    ps1 = ps1p.tile([128, HID], F32, name="ps1")
                    o = (blk - b0) * 128
                    for k in range(IN // 128):
                        b.tensor.matmul(
                            ps1[:, :], lhsT=xT_s[:, k, o:o + 128],
                            rhs=W1_s[:, k, :], start=(k == 0),
                            stop=(k == IN // 128 - 1))
                    b.vector.tensor_scalar_mul(
                        out=hbf[:, blk, :], in0=ps1[:, :],
                        scalar1=dvo_s[:, blk:blk + 1])
                # stage this chunk's rows to DRAM while later chunks compute
                bf = min(b1_, NFULL)
                if bf > b0:
                    b.sync.dma_start(
                        agin1[b0 * 128:bf * 128, :].rearrange(
                            "(c p) f -> p c f", p=128),
                        hbf[:, b0:bf, :])
                if NREM and b1_ == NB128:
                    b.sync.dma_start(agin1[NFULL * 128:, :],
                                     hbf[:NREM, NFULL, :])
        b.gpsimd.collective_compute(
            "AllGather", mybir.AluOpType.bypass,
            replica_groups=[list(range(ncores))],
            ins=[agin1[:, :].opt()], outs=[h_full[:, :].opt()])

        # ---- SpMM machinery
        def spmm(feat_full, F, gdt, mdt, id_s, hown, hT_out, bias_s, layer,
                 on_block=None):
            """Aggregate feat_full (DRAM [n, F]) into
            hT_out [128, F//128, NBLK, BLK] = Relu(psum * dinv_dst + bias),
            features on partitions. hown is this core's own scaled-h tile
            ([128, NB128, F], node-major) used for the self-loop term."""
            nfc = F // 128
            streams = {0: feat_full[0:HALF, :], 1: feat_full[HALF:n, :]}
            gpool = tc.alloc_tile_pool(name=f"g{layer}", bufs=GBUFS)
            mpool = tc.alloc_tile_pool(name=f"m{layer}", bufs=4)
            pspool = tc.alloc_tile_pool(name=f"ps{layer}", bufs=4, space="PSUM")
            sbpool = tc.alloc_tile_pool(name=f"sb{layer}", bufs=4)

            dbg_done = [False, False]

            mstate = {}
            def ensure_m(mc):
                if mc in mstate:
                    return mstate[mc]
                t0 = mc * MCHUNK
                ntm = min(MCHUNK, NT - t0)
                mt = mpool.tile([128, MCHUNK, BLK], mdt, name=f"mt{layer}")
                b.vector.tensor_tensor(
                    out=mt[:, :ntm, :], in0=iota_s[:, :ntm, :],
                    in1=dl_s[:, t0:t0 + ntm].to_broadcast([128, ntm, BLK]),
                    op=mybir.AluOpType.is_equal)
                mstate.clear()
                mstate[mc] = mt
                if DEBUG and layer == 1 and not dbg_done[0]:
                    dbg_done[0] = True
                    b.sync.dma_start(
                        dbg_m.ap().rearrange("p (c k) -> p c k", k=BLK),
                        mt[:, :, :])
                return mt

            idx_tabs = {0: il_s, 1: ih_s}
            ntiles_h = {0: TL, 1: TH}
            gmap = {0: {}, 1: {}}
            gcnt = {0: 0, 1: 0}

            def ensure_gather(s, th):
                while th >= gcnt[s] * MAXCT:
                    c0 = gcnt[s] * MAXCT
                    nt = min(MAXCT, ntiles_h[s] - c0)
                    gt = gpool.tile([128, MAXCT, F], gdt, name=f"gt{layer}{s}")
                    b.gpsimd.dma_gather(
                        out_ap=gt[:, :nt, :],
                        in_ap=streams[s],
                        idxs_ap=idx_tabs[s][:, c0 * 8:(c0 + nt) * 8],
                        num_idxs=nt * TILE,
                        num_idxs_reg=nt * TILE,
                        elem_size=F,
                        single_packet=True,
                        queue_num=(s * 2 + gcnt[s] % 2),
                    )
                    gmap[s][gcnt[s]] = gt
                    gcnt[s] += 1

            g = 0
            cons = {0: 0, 1: 0}
            for blk in range(NBLK):
                tot = sched[blk][0] + sched[blk][1]
                ps = [pspool.tile([128, BLK], F32, name=f"ps{layer}_{fc}")
                      for fc in range(nfc)]
                for fc in range(nfc):
                    b.tensor.matmul(
                        ps[fc][:, :], lhsT=hown[:, blk, fc * 128:(fc + 1) * 128],
                        rhs=id_s[:, :], start=True, stop=(tot == 0))
                ti_blk = 0
                for h in (0, 1):
                    for _ in range(sched[blk][h]):
                        th = cons[h]
                        ensure_gather(h, th)
                        gt = gmap[h][th // MAXCT]
                        gc = th % MAXCT
                        mt = ensure_m(g // MCHUNK)
                        mo = g % MCHUNK
                        last = (ti_blk == tot - 1)
                        for fc in range(nfc):
                            b.tensor.matmul(
                                ps[fc][:, :],
                                lhsT=gt[:, gc, fc * 128:(fc + 1) * 128],
                                rhs=mt[:, mo, :],
                                start=False, stop=last)
                        cons[h] += 1
                        g += 1
                        ti_blk += 1
                for fc in range(nfc):
                    tmp = sbpool.tile([128, BLK], BF, name=f"tp{layer}_{fc}")
                    b.vector.tensor_tensor(
                        out=tmp[:, :], in0=ps[fc][:, :], in1=dvd_s[:, blk, :],
                        op=mybir.AluOpType.mult)
                    b.scalar.activation(
                        out=hT_out[:, fc, blk, :], in_=tmp[:, :],
                        func=mybir.ActivationFunctionType.Relu,
                        bias=bias_s[:, fc:fc + 1])
                if on_block is not None:
                    on_block(blk)
            assert g == NT and cons[0] == TL and cons[1] == TH
            for p in (sbpool, pspool, mpool, gpool):
                p.release()

        # ---- layer 1 SpMM -> h1T (hbf stays alive for the self term)
        z2bf, z2bf_free = tc.tile([128, NB128, MID], BF, name="z2bf")
        h1T, h1T_free = tc.tile([128, HID // 128, NBLK, BLK], BF, name="h1T")
        if DEBUG:
            b.sync.dma_start(dbg_hb.ap().rearrange("p (c k) -> p c k", k=HID),
                             hbf[:, :, :])
            b.sync.dma_start(dbg_hf.ap(), h_full[:, :])
        spmm(h_full, HID, FP8, FP8, id8_s, hbf, h1T, b1_s, layer=1)
        if DEBUG:
            b.sync.dma_start(
                dbg_h1.ap().rearrange("p (a c k) -> p a c k", c=NBLK, k=BLK),
                h1T[:, :, :, :])

        # ---- z2 = dinv * (h1 @ W2), AllGather (staged to DRAM per chunk)
        with tc.tile_pool(name="ps2p", bufs=2, space="PSUM") as ps2p:
            for xc in range(NXC):
                b0, b1_ = xc * CBLK, min((xc + 1) * CBLK, NB128)
                for blk in range(b0, b1_):
                    psz = ps2p.tile([128, MID], F32, name="psz")
                    for fc in range(HID // 128):
                        b.tensor.matmul(
                            psz[:, :],
                            lhsT=h1T[:, fc, blk, :],
                            rhs=W2_s[:, fc, :],
                            start=(fc == 0), stop=(fc == HID // 128 - 1))
                    b.vector.tensor_scalar_mul(
                        out=z2bf[:, blk, :], in0=psz[:, :],
                        scalar1=dvo_s[:, blk:blk + 1])
                bf = min(b1_, NFULL)
                if bf > b0:
                    b.sync.dma_start(
                        agin2[b0 * 128:bf * 128, :].rearrange(
                            "(c p) f -> p c f", p=128),
                        z2bf[:, b0:bf, :])
                if NREM and b1_ == NB128:
                    b.sync.dma_start(agin2[NFULL * 128:, :],
                                     z2bf[:NREM, NFULL, :])
        h1T_free()
        b.gpsimd.collective_compute(
            "AllGather", mybir.AluOpType.bypass,
            replica_groups=[list(range(ncores))],
            ins=[agin2[:, :].opt()], outs=[z2_full[:, :].opt()])

        # ---- layer 2 SpMM -> h2T (z2bf stays alive for the self term)
        h2T, h2T_free = tc.tile([128, MID // 128, NBLK, BLK], BF, name="h2T")
        spmm(z2_full, MID, BF, BF, id16_s, z2bf, h2T, b2_s, layer=2)

        # ---- logits + log_softmax
        ls_s, ls_s_free = tc.tile([128, NB128, OUT], F32, name="lss")
        ex_s, ex_s_free = tc.tile([128, NB128, OUT], F32, name="exs")
        mx_s, mx_s_free = tc.tile([128, NB128], F32, name="mxs")
        sm_s, sm_s_free = tc.tile([128, NB128], F32, name="sms")
        lg_s, lg_s_free = tc.tile([128, NB128], F32, name="lgs")
        with tc.tile_pool(name="ps3p", bufs=2, space="PSUM") as ps3p:
            for blk in range(NB128):
                psl = ps3p.tile([128, OUT], F32, name="psl")
                b.tensor.matmul(
                    psl[:, :],
                    lhsT=h2T[:, 0, blk, :],
                    rhs=Wc_s[:, :], start=True, stop=True)
                b.vector.tensor_tensor(out=ls_s[:, blk, :], in0=psl[:, :],
                                       in1=bc_s[:, :], op=mybir.AluOpType.add)
        b.vector.reduce_max(out=mx_s[:, :], in_=ls_s[:, :, :],
                            axis=mybir.AxisListType.X)
        b.vector.tensor_tensor(out=ls_s[:, :, :], in0=ls_s[:, :, :],
                               in1=mx_s[:, :].to_broadcast([128, NB128, OUT]),
                               op=mybir.AluOpType.subtract)
        b.scalar.activation(out=ex_s[:, :, :], in_=ls_s[:, :, :],
                            func=mybir.ActivationFunctionType.Exp)
        b.vector.reduce_sum(out=sm_s[:, :], in_=ex_s[:, :, :],
                            axis=mybir.AxisListType.X)
        b.scalar.activation(out=lg_s[:, :], in_=sm_s[:, :],
                            func=mybir.ActivationFunctionType.Ln)
        b.vector.tensor_tensor(out=ex_s[:, :, :], in0=ls_s[:, :, :],
                               in1=lg_s[:, :].to_broadcast([128, NB128, OUT]),
                               op=mybir.AluOpType.subtract)
        b.sync.dma_start(out_d.ap()[:NFULL * 128, :].rearrange(
            "(c p) f -> p c f", p=128), ex_s[:, :NFULL, :])
        if NREM:
            b.sync.dma_start(out_d.ap()[NFULL * 128:, :], ex_s[:NREM, NFULL, :])

        # release remaining pools in LIFO order (leftover TilePoolBoundary
        # pseudo-ops crash walrus)
        lg_s_free(); sm_s_free(); mx_s_free(); ex_s_free(); ls_s_free()
        h2T_free()
        z2bf_free()
        hbf_free()
        dshp.release()
        dvd_s_free(); dvo_s_free()
        id16_s_free(); id8_s_free(); iota_s_free()
        dl_s_free(); ih_s_free(); il_s_free()
        bc_s_free(); b2_s_free(); b1_s_free()
        Wc_s_free(); W2_s_free(); W1_s_free()

    return b


# ---------------------------------------------------------------- entry point

def make_in_maps(inputs, p, n=None, ncores=NC):
    if n is None:
        n = N
    d = p['d']
    NPC, NB128 = d['NPC'], d['NB128']
    NPAD = NB128 * 128
    NBLK = NB128
    x = np.asarray(inputs['x'], np.float32)
    W1 = np.asarray(inputs['W1'], np.float32).astype(BF16NP)
    W2 = np.asarray(inputs['W2'], np.float32).astype(BF16NP)
    Wc = np.asarray(inputs['Wc'], np.float32).astype(BF16NP)
    b1 = np.asarray(inputs['b1'], np.float32)
    b2 = np.asarray(inputs['b2'], np.float32)
    bc = np.asarray(inputs['bc'], np.float32)
    dinv = p['dinv']

    b1c = b1.reshape(HID // 128, 128).T.copy()
    b2c = b2.reshape(MID // 128, 128).T.copy()
    bcr = np.tile(bc[None, :], (128, 1))
    iota = np.tile(np.arange(BLK, dtype=np.float32)[None, :],
                   (128, MCHUNK)).astype(BF16NP)
    id8 = np.eye(128, dtype=np.float32).astype(FP8NP if USE_FP8 else BF16NP)
    id16 = np.eye(128, dtype=np.float32).astype(BF16NP)

    in_maps = []
    for c in range(ncores):
        rows = slice(c * NPC, (c + 1) * NPC)
        xT = np.zeros((IN, NPAD), BF16NP)
        xT[:, :NPC] = x[rows].T.astype(BF16NP)
        dv = dinv[rows]
        tmpv = np.zeros(NB128 * 128, np.float32)
        tmpv[:NPC] = dv
        dvo = tmpv.reshape(NB128, 128).T.copy()      # dinv by (row%128, row//128)
        dvd = np.zeros((128, NBLK * BLK), BF16NP)    # dinv[dst] replicated /parts
        dvd[:, :NPC] = np.tile(dv.astype(BF16NP)[None, :], (128, 1))
        pc = p['per_core'][c]
        in_maps.append(dict(
            xT=xT, W1=W1, W2=W2, Wc=Wc, b1c=b1c, b2c=b2c, bcr=bcr,
            idx_lo=pc['idx_lo'], idx_hi=pc['idx_hi'], dl=pc['dl'],
            iota=iota, id8=id8, id16=id16, dvo=dvo, dvd=dvd))
    return in_maps


def kernel_with_results(inputs, trace=False, **kw):
    from concourse import bacc
    from concourse import bass_utils

    edge_index = np.asarray(inputs['edge_index'])
    p = plan(edge_index, n=N)
    nc_obj = bacc.Bacc("TRN2", target_bir_lowering=False, debug=False,
                       num_devices=NC, num_swdge_queues=4)
    build(nc_obj, p['sched'], p['NT'], p['TL'], p['TH'], n=N)
    nc_obj.compile()
    in_maps = make_in_maps(inputs, p, n=N)
    res = bass_utils.run_bass_kernel_spmd(nc_obj, in_maps,
                                          core_ids=list(range(NC)),
                                          trace=trace, **kw)
    out = np.concatenate([np.asarray(res.results[c]['out']) for c in range(NC)],
                         axis=0)
    return out.astype(np.float32), res


def kernel(**inputs) -> np.ndarray:
    return kernel_with_results(inputs)[0]


if __name__ == '__main__':
    import reference
    inputs = {k: np.asarray(v) for k, v in reference.setup_inputs().items()}
    out = kernel(**inputs)
    print('kernel out', out.shape, out.dtype)


# revision 55
# speedup vs baseline: 1.1583x; 1.0026x over previous
"""GCN (2x GCNConv + classifier + log_softmax) on 8 Trainium2 NeuronCores.

Strategy (node sharding, per-core c owns rows [c*NPC, (c+1)*NPC)):
  h  = dinv * (x @ W1)      dense per-core rows from host-pre-transposed xT
                            (bf16); the src-side degree norm is folded into the
                            PSUM->fp8 cast. AllGather h (fp8e4).
  agg1 = Ahat @ h           dma_gather rows + one-hot matmul into PSUM per
                            128-dst block; the one-hot M is generated ON-CHIP
                            (DVE is_equal against an iota constant) from a tiny
                            dst-local table; self-loop terms are injected with
                            an identity-matmul of the core's own h block (no
                            gather, no virtual edges). flush = psum * dinv[dst]
                            (DVE) then h1T = Relu(+b1) on ScalarE, transposed
                            layout (features on partitions).
  z2 = dinv * (h1 @ W2)     dense from h1T; AllGather z2 (bf16)
  agg2 = Ahat @ z2          same SpMM -> h2T = Relu(psum*dinv[dst] + b2)
  logits = h2 @ Wc + bc     dense from h2T; batched log_softmax on free axis

Edge handling (host planner): edges are partitioned by dst-owner core, split
into lo/hi streams by src half (dma_gather int16 index limit), grouped by
128-dst block into 128-edge tiles (tile count per (block, half) = max across
cores so one SPMD program serves every core; pad slots gather row 0 and have
dl=-1, giving zero M columns). Gather calls chain 8 full tiles through each
half's tile stream - per-call GpSimd issue cost dominates, so few full calls
beat many per-cell fragments, and beat per-core pad skipping via num_idxs_reg.
"""
import sys
import numpy as np

sys.path.insert(0, '/opt/trn_rl_repo')

import ml_dtypes

BF16NP = ml_dtypes.bfloat16
FP8NP = ml_dtypes.float8_e4m3fn

USE_FP8 = True       # store/gather h in fp8e4 (layer 1); bf16 otherwise
DEBUG = False        # add intermediate-dump DRAM outputs
# Gather calls chain 8 full tiles from each src-half stream (GpSimd per-call
# fixed cost dominates fragmented per-cell calls). Pad slots gather row 0:
# skipping them per-core via num_idxs_reg works but forces per-cell call
# fragmentation, which costs more GpSimd issue time than the pads cost DMA.

N, E_EXPECT, IN, HID, MID, OUT = 50000, 800000, 512, 256, 128, 64
NC = 8
TILE = 128           # edges per matmul tile (contraction dim)
BLK = 128            # dst nodes per PSUM accumulation block
MAXCT = 8            # max tiles per dma_gather call (1024 rows; >1024 hard-
                     # crashes single-packet SWDGE on HW - verified)
MCHUNK = 8           # tiles per on-chip M generation chunk (DVE is_equal)
GBUFS = 16           # gather pool buffers


def _derived(n=None, nc=NC):
    if n is None:
        n = N
    npc = n // nc
    return dict(
        NPC=npc,
        NB128=(npc + 127) // 128,         # 128-row blocks (== SpMM dst blocks)
        HALF=n // 2,
    )


# ---------------------------------------------------------------- host planner

def plan(edge_index: np.ndarray, n=None, nc=NC):
    if n is None:
        n = N
    d = _derived(n, nc)
    npc, half = d['NPC'], d['HALF']
    nblk = d['NB128']

    src = edge_index[0].astype(np.int64)
    dst = edge_index[1].astype(np.int64)
    deg = np.ones(n, np.float64)
    np.add.at(deg, dst, 1.0)
    dinv = (1.0 / np.sqrt(deg)).astype(np.float32)

    core = dst // npc
    halfof = (src >= half).astype(np.int64)
    block = (dst % npc) // BLK
    dloc = (dst % npc) % BLK

    counts = np.zeros((nc, nblk, 2), np.int64)
    np.add.at(counts, (core, block, halfof), 1)
    maxcnt = counts.max(axis=0)               # [nblk, 2]

    # static schedule: tiles per (block, half) cell; gather calls chain 8
    # tiles at a time through each half's tile stream (block-major).
    sched = [[int(-(-maxcnt[b, h] // TILE)) for h in (0, 1)]
             for b in range(nblk)]
    TL = sum(s[0] for s in sched)
    TH = sum(s[1] for s in sched)
    NT = TL + TH
    # global (M/dl) tile index: block-major, lo tiles then hi tiles per block
    gbase = {}
    g = 0
    for b in range(nblk):
        for h in (0, 1):
            gbase[(b, h)] = g
            g += sched[b][h]

    def gather_layout(flat_idx):
        """[T*128] int -> [128, T*8] int16 in dma_gather layout, per 8-tile
        call chunks."""
        ntiles = len(flat_idx) // TILE
        cols = np.zeros((16, ntiles * 8), np.int16)
        pos = 0
        for c0 in range(0, ntiles, MAXCT):
            nt = min(MAXCT, ntiles - c0)
            nidx = nt * TILE
            cols[:, c0 * 8:c0 * 8 + nt * 8] = \
                flat_idx[pos:pos + nidx].reshape(-1, 16).T.astype(np.int16)
            pos += nidx
        return np.tile(cols, (8, 1))

    per_core = []
    for c in range(nc):
        m = core == c
        csrc, chalf = src[m], halfof[m]
        cblock, cdloc = block[m], dloc[m]
        dl_tab = np.full((128, NT), -1.0, np.float32)
        streams = {}
        for h in (0, 1):
            parts = []
            for b in range(nblk):
                tiles = sched[b][h]
                if tiles == 0:
                    continue
                mask = (cblock == b) & (chalf == h)
                sr = csrc[mask] - (half if h else 0)
                dls = cdloc[mask]
                ne = len(sr)
                cap = tiles * TILE
                assert ne <= cap
                ii = np.zeros(cap, np.int64)          # pads gather row 0
                ddl = np.full(cap, -1, np.int64)
                ii[:ne] = sr
                ddl[:ne] = dls
                parts.append(ii)
                g0 = gbase[(b, h)]
                for ti in range(tiles):
                    dl_tab[:, g0 + ti] = ddl[ti * TILE:(ti + 1) * TILE]
            streams[h] = gather_layout(np.concatenate(parts))
        per_core.append(dict(
            idx_lo=streams[0], idx_hi=streams[1],
            dl=dl_tab.astype(BF16NP)))

    return dict(sched=sched, NT=NT, TL=TL, TH=TH, per_core=per_core,
                dinv=dinv, d=d)


# ---------------------------------------------------------------- bass program

def build(nc_obj, sched, NT, TL, TH, n=None, ncores=NC):
    if n is None:
        n = N
    from concourse import bass, mybir, tile

    BF = mybir.dt.bfloat16
    F32 = mybir.dt.float32
    I16 = mybir.dt.int16
    FP8 = mybir.dt.float8e4 if USE_FP8 else BF

    d = _derived(n, ncores)
    NPC, NB128, HALF = d['NPC'], d['NB128'], d['HALF']
    NBLK = NB128
    NPAD = NB128 * 128
    NFULL = NPC // 128            # full 128-row blocks
    NREM = NPC - NFULL * 128      # rows in the final partial block

    b = nc_obj
    xT_d = b.declare_dram_parameter("xT", [IN, NPAD], BF, isOutput=False)
    W1_d = b.declare_dram_parameter("W1", [IN, HID], BF, isOutput=False)
    W2_d = b.declare_dram_parameter("W2", [HID, MID], BF, isOutput=False)
    Wc_d = b.declare_dram_parameter("Wc", [MID, OUT], BF, isOutput=False)
    b1_d = b.declare_dram_parameter("b1c", [128, HID // 128], F32, isOutput=False)
    b2_d = b.declare_dram_parameter("b2c", [128, MID // 128], F32, isOutput=False)
    bc_d = b.declare_dram_parameter("bcr", [128, OUT], F32, isOutput=False)
    il_d = b.declare_dram_parameter("idx_lo", [128, TL * 8], I16, isOutput=False)
    ih_d = b.declare_dram_parameter("idx_hi", [128, TH * 8], I16, isOutput=False)
    dl_d = b.declare_dram_parameter("dl", [128, NT], BF, isOutput=False)
    iota_d = b.declare_dram_parameter("iota", [128, MCHUNK * BLK], BF,
                                      isOutput=False)
    id8_d = b.declare_dram_parameter("id8", [128, 128], FP8, isOutput=False)
    id16_d = b.declare_dram_parameter("id16", [128, 128], BF, isOutput=False)
    dvo_d = b.declare_dram_parameter("dvo", [128, NB128], F32, isOutput=False)
    dvd_d = b.declare_dram_parameter("dvd", [128, NBLK * BLK], BF, isOutput=False)
    out_d = b.declare_dram_parameter("out", [NPC, OUT], F32, isOutput=True)
    if DEBUG:
        dbg_hb = b.declare_dram_parameter("dbg_hb", [128, NB128 * HID], FP8,
                                          isOutput=True)
        dbg_m = b.declare_dram_parameter("dbg_m", [128, MCHUNK * BLK], FP8,
                                         isOutput=True)
        dbg_g = b.declare_dram_parameter("dbg_g", [128, MAXCT * HID], FP8,
                                         isOutput=True)
        dbg_h1 = b.declare_dram_parameter("dbg_h1", [128, (HID // 128) * NBLK * BLK],
                                          BF, isOutput=True)
        dbg_hf = b.declare_dram_parameter("dbg_hf", [n, HID], FP8, isOutput=True)

    with tile.TileContext(b) as tc:
        # ---- resident SBUF tables
        W1_s, W1_s_free = tc.tile([128, IN // 128, HID], BF, name="W1s")
        W2_s, W2_s_free = tc.tile([128, HID // 128, MID], BF, name="W2s")
        Wc_s, Wc_s_free = tc.tile([128, OUT], BF, name="Wcs")
        b1_s, b1_s_free = tc.tile([128, HID // 128], F32, name="b1s")
        b2_s, b2_s_free = tc.tile([128, MID // 128], F32, name="b2s")
        bc_s, bc_s_free = tc.tile([128, OUT], F32, name="bcs")
        il_s, il_s_free = tc.tile([128, TL * 8], I16, name="ils")
        ih_s, ih_s_free = tc.tile([128, TH * 8], I16, name="ihs")
        dl_s, dl_s_free = tc.tile([128, NT], BF, name="dls")
        iota_s, iota_s_free = tc.tile([128, MCHUNK, BLK], BF, name="iotas")
        id8_s, id8_s_free = tc.tile([128, 128], FP8, name="id8s")
        id16_s, id16_s_free = tc.tile([128, 128], BF, name="id16s")
        dvo_s, dvo_s_free = tc.tile([128, NB128], F32, name="dvos")
        dvd_s, dvd_s_free = tc.tile([128, NBLK, BLK], BF, name="dvds")
        b.sync.dma_start(W1_s[:, :, :], W1_d.ap().rearrange("(k p) h -> p k h", p=128))
        b.sync.dma_start(W2_s[:, :, :], W2_d.ap().rearrange("(k p) h -> p k h", p=128))
        b.sync.dma_start(Wc_s[:, :], Wc_d.ap())
        b.sync.dma_start(b1_s[:, :], b1_d.ap())
        b.sync.dma_start(b2_s[:, :], b2_d.ap())
        b.sync.dma_start(bc_s[:, :], bc_d.ap())
        b.sync.dma_start(il_s[:, :], il_d.ap())
        b.sync.dma_start(ih_s[:, :], ih_d.ap())
        b.sync.dma_start(dl_s[:, :], dl_d.ap())
        b.sync.dma_start(iota_s[:, :, :],
                         iota_d.ap().rearrange("p (c k) -> p c k", k=BLK))
        b.sync.dma_start(id8_s[:, :], id8_d.ap())
        b.sync.dma_start(id16_s[:, :], id16_d.ap())
        b.sync.dma_start(dvo_s[:, :], dvo_d.ap())
        b.sync.dma_start(dvd_s[:, :, :],
                         dvd_d.ap().rearrange("p (c k) -> p c k", k=BLK))

        # DRAM staging (persist; declared first so SBUF frees stay LIFO).
        dshp = tc.alloc_tile_pool(name="dsh", bufs=1, space="DRAM")
        agin1 = dshp.tile([NPC, HID], FP8, name="agin1")
        h_full = dshp.tile([n, HID], FP8, name="h_full", addr_space="Shared")
        agin2 = dshp.tile([NPC, MID], BF, name="agin2")
        z2_full = dshp.tile([n, MID], BF, name="z2_full", addr_space="Shared")

        # ---- phase 1: h = dinv * (x @ W1), cast fp8, AllGather
        # xT is loaded in node-range chunks so the dense matmuls of chunk 0
        # overlap the load of chunk 1.
        hbf, hbf_free = tc.tile([128, NB128, HID], FP8, name="hbf")
        NXC = 4                               # node chunks
        CBLK = -(-NB128 // NXC)               # blocks per chunk
        with (tc.tile_pool(name="xtp", bufs=2) as xtp,
              tc.tile_pool(name="ps1p", bufs=2, space="PSUM") as ps1p):
            xT_r = xT_d.ap().rearrange("(k p) n -> p k n", p=128)
            for xc in range(NXC):
                b0, b1_ = xc * CBLK, min((xc + 1) * CBLK, NB128)
                xT_s = xtp.tile([128, IN // 128, CBLK * 128], BF, name="xTs")
                b.sync.dma_start(xT_s[:, :, :(b1_ - b0) * 128],
                                 xT_r[:, :, b0 * 128:b1_ * 128])
                for blk in range(b0, b1_):
                    ps1 = ps1p.tile([128, HID], F32, name="ps1")
                    o = (blk - b0) * 128
                    for k in range(IN // 128):
                        b.tensor.matmul(
                            ps1[:, :], lhsT=xT_s[:, k, o:o + 128],
                            rhs=W1_s[:, k, :], start=(k == 0),
                            stop=(k == IN // 128 - 1))
                    b.vector.tensor_scalar_mul(
                        out=hbf[:, blk, :], in0=ps1[:, :],
                        scalar1=dvo_s[:, blk:blk + 1])
                # stage this chunk's rows to DRAM while later chunks compute
                bf = min(b1_, NFULL)
                if bf > b0:
                    b.sync.dma_start(
                        agin1[b0 * 128:bf * 128, :].rearrange(
                            "(c p) f -> p c f", p=128),
                        hbf[:, b0:bf, :])
                if NREM and b1_ == NB128:
                    b.sync.dma_start(agin1[NFULL * 128:, :],
                                     hbf[:NREM, NFULL, :])
        b.gpsimd.collective_compute(
            "AllGather", mybir.AluOpType.bypass,
            replica_groups=[list(range(ncores))],
            ins=[agin1[:, :].opt()], outs=[h_full[:, :].opt()])

        # ---- SpMM machinery
        def spmm(feat_full, F, gdt, mdt, id_s, hown, hT_out, bias_s, layer,
                 on_block=None):
            """Aggregate feat_full (DRAM [n, F]) into
            hT_out [128, F//128, NBLK, BLK] = Relu(psum * dinv_dst + bias),
            features on partitions. hown is this core's own scaled-h tile
            ([128, NB128, F], node-major) used for the self-loop term."""
            nfc = F // 128
            streams = {0: feat_full[0:HALF, :], 1: feat_full[HALF:n, :]}
            gpool = tc.alloc_tile_pool(name=f"g{layer}", bufs=GBUFS)
            mpool = tc.alloc_tile_pool(name=f"m{layer}", bufs=4)
            pspool = tc.alloc_tile_pool(name=f"ps{layer}", bufs=4, space="PSUM")
            sbpool = tc.alloc_tile_pool(name=f"sb{layer}", bufs=4)

            dbg_done = [False, False]

            mstate = {}
            def ensure_m(mc):
                if mc in mstate:
                    return mstate[mc]
                t0 = mc * MCHUNK
                ntm = min(MCHUNK, NT - t0)
                mt = mpool.tile([128, MCHUNK, BLK], mdt, name=f"mt{layer}")
                b.vector.tensor_tensor(
                    out=mt[:, :ntm, :], in0=iota_s[:, :ntm, :],
                    in1=dl_s[:, t0:t0 + ntm].to_broadcast([128, ntm, BLK]),
                    op=mybir.AluOpType.is_equal)
                mstate.clear()
                mstate[mc] = mt
                if DEBUG and layer == 1 and not dbg_done[0]:
                    dbg_done[0] = True
                    b.sync.dma_start(
                        dbg_m.ap().rearrange("p (c k) -> p c k", k=BLK),
                        mt[:, :, :])
                return mt

            idx_tabs = {0: il_s, 1: ih_s}
            ntiles_h = {0: TL, 1: TH}
            gmap = {0: {}, 1: {}}
            gcnt = {0: 0, 1: 0}

            def ensure_gather(s, th):
                while th >= gcnt[s] * MAXCT:
                    c0 = gcnt[s] * MAXCT
                    nt = min(MAXCT, ntiles_h[s] - c0)
                    gt = gpool.tile([128, MAXCT, F], gdt, name=f"gt{layer}{s}")
                    b.gpsimd.dma_gather(
                        out_ap=gt[:, :nt, :],
                        in_ap=streams[s],
                        idxs_ap=idx_tabs[s][:, c0 * 8:(c0 + nt) * 8],
                        num_idxs=nt * TILE,
                        num_idxs_reg=nt * TILE,
                        elem_size=F,
                        single_packet=True,
                        queue_num=(s * 2 + gcnt[s] % 2),
                    )
                    gmap[s][gcnt[s]] = gt
                    gcnt[s] += 1

            g = 0
            cons = {0: 0, 1: 0}
            for blk in range(NBLK):
                tot = sched[blk][0] + sched[blk][1]
                ps = [pspool.tile([128, BLK], F32, name=f"ps{layer}_{fc}")
                      for fc in range(nfc)]
                for fc in range(nfc):
                    b.tensor.matmul(
                        ps[fc][:, :], lhsT=hown[:, blk, fc * 128:(fc + 1) * 128],
                        rhs=id_s[:, :], start=True, stop=(tot == 0))
                ti_blk = 0
                for h in (0, 1):
                    for _ in range(sched[blk][h]):
                        th = cons[h]
                        ensure_gather(h, th)
                        gt = gmap[h][th // MAXCT]
                        gc = th % MAXCT
                        mt = ensure_m(g // MCHUNK)
                        mo = g % MCHUNK
                        last = (ti_blk == tot - 1)
                        for fc in range(nfc):
                            b.tensor.matmul(
                                ps[fc][:, :],
                                lhsT=gt[:, gc, fc * 128:(fc + 1) * 128],
                                rhs=mt[:, mo, :],
                                start=False, stop=last)
                        cons[h] += 1
                        g += 1
                        ti_blk += 1
                for fc in range(nfc):
                    tmp = sbpool.tile([128, BLK], BF, name=f"tp{layer}_{fc}")
                    b.vector.tensor_tensor(
                        out=tmp[:, :], in0=ps[fc][:, :], in1=dvd_s[:, blk, :],
                        op=mybir.AluOpType.mult)
                    b.scalar.activation(
                        out=hT_out[:, fc, blk, :], in_=tmp[:, :],
                        func=mybir.ActivationFunctionType.Relu,
                        bias=bias_s[:, fc:fc + 1])
                if on_block is not None:
                    on_block(blk)
            assert g == NT and cons[0] == TL and cons[1] == TH
            for p in (sbpool, pspool, mpool, gpool):
                p.release()

        # ---- layer 1 SpMM -> h1T (hbf stays alive for the self term)
        z2bf, z2bf_free = tc.tile([128, NB128, MID], BF, name="z2bf")
        h1T, h1T_free = tc.tile([128, HID // 128, NBLK, BLK], BF, name="h1T")
        if DEBUG:
            b.sync.dma_start(dbg_hb.ap().rearrange("p (c k) -> p c k", k=HID),
                             hbf[:, :, :])
            b.sync.dma_start(dbg_hf.ap(), h_full[:, :])
        spmm(h_full, HID, FP8, FP8, id8_s, hbf, h1T, b1_s, layer=1)
        if DEBUG:
            b.sync.dma_start(
                dbg_h1.ap().rearrange("p (a c k) -> p a c k", c=NBLK, k=BLK),
                h1T[:, :, :, :])

        # ---- z2 = dinv * (h1 @ W2), AllGather (staged to DRAM per chunk)
        with tc.tile_pool(name="ps2p", bufs=2, space="PSUM") as ps2p:
            for xc in range(NXC):
                b0, b1_ = xc * CBLK, min((xc + 1) * CBLK, NB128)
                for blk in range(b0, b1_):
                    psz = ps2p.tile([128, MID], F32, name="psz")
                    for fc in range(HID // 128):
                        b.tensor.matmul(
                            psz[:, :],
                            lhsT=h1T[:, fc, blk, :],
                            rhs=W2_s[:, fc, :],
                            start=(fc == 0), stop=(fc == HID // 128 - 1))
                    b.vector.tensor_scalar_mul(
                        out=z2bf[:, blk, :], in0=psz[:, :],
                        scalar1=dvo_s[:, blk:blk + 1])
                bf = min(b1_, NFULL)
                if bf > b0:
                    b.sync.dma_start(
                        agin2[b0 * 128:bf * 128, :].rearrange(
                            "(c p) f -> p c f", p=128),
                        z2bf[:, b0:bf, :])
                if NREM and b1_ == NB128:
                    b.sync.dma_start(agin2[NFULL * 128:, :],
                                     z2bf[:NREM, NFULL, :])
        h1T_free()
        b.gpsimd.collective_compute(
            "AllGather", mybir.AluOpType.bypass,
            replica_groups=[list(range(ncores))],
            ins=[agin2[:, :].opt()], outs=[z2_full[:, :].opt()])

        # ---- layer 2 SpMM -> h2T (z2bf stays alive for the self term)
        h2T, h2T_free = tc.tile([128, MID // 128, NBLK, BLK], BF, name="h2T")
        spmm(z2_full, MID, BF, BF, id16_s, z2bf, h2T, b2_s, layer=2)

        # ---- logits + log_softmax
        ls_s, ls_s_free = tc.tile([128, NB128, OUT], F32, name="lss")
        ex_s, ex_s_free = tc.tile([128, NB128, OUT], F32, name="exs")
        mx_s, mx_s_free = tc.tile([128, NB128], F32, name="mxs")
        sm_s, sm_s_free = tc.tile([128, NB128], F32, name="sms")
        lg_s, lg_s_free = tc.tile([128, NB128], F32, name="lgs")
        with tc.tile_pool(name="ps3p", bufs=2, space="PSUM") as ps3p:
            for blk in range(NB128):
                psl = ps3p.tile([128, OUT], F32, name="psl")
                b.tensor.matmul(
                    psl[:, :],
                    lhsT=h2T[:, 0, blk, :],
                    rhs=Wc_s[:, :], start=True, stop=True)
                b.vector.tensor_tensor(out=ls_s[:, blk, :], in0=psl[:, :],
                                       in1=bc_s[:, :], op=mybir.AluOpType.add)
        b.vector.reduce_max(out=mx_s[:, :], in_=ls_s[:, :, :],
                            axis=mybir.AxisListType.X)
        b.vector.tensor_tensor(out=ls_s[:, :, :], in0=ls_s[:, :, :],
                               in1=mx_s[:, :].to_broadcast([128, NB128, OUT]),
                               op=mybir.AluOpType.subtract)
        b.scalar.activation(out=ex_s[:, :, :], in_=ls_s[:, :, :],
                            func=mybir.ActivationFunctionType.Exp)
        b.vector.reduce_sum(out=sm_s[:, :], in_=ex_s[:, :, :],
                            axis=mybir.AxisListType.X)
        b.scalar.activation(out=lg_s[:, :], in_=sm_s[:, :],
                            func=mybir.ActivationFunctionType.Ln)
        b.vector.tensor_tensor(out=ex_s[:, :, :], in0=ls_s[:, :, :],
                               in1=lg_s[:, :].to_broadcast([128, NB128, OUT]),
                               op=mybir.AluOpType.subtract)
        b.sync.dma_start(out_d.ap()[:NFULL * 128, :].rearrange(
            "(c p) f -> p c f", p=128), ex_s[:, :NFULL, :])
        if NREM:
            b.sync.dma_start(out_d.ap()[NFULL * 128:, :], ex_s[:NREM, NFULL, :])

        # release remaining pools in LIFO order (leftover TilePoolBoundary
        # pseudo-ops crash walrus)
        lg_s_free(); sm_s_free(); mx_s_free(); ex_s_free(); ls_s_free()
        h2T_free()
        z2bf_free()
        hbf_free()
        dshp.release()
        dvd_s_free(); dvo_s_free()
        id16_s_free(); id8_s_free(); iota_s_free()
        dl_s_free(); ih_s_free(); il_s_free()
        bc_s_free(); b2_s_free(); b1_s_free()
        Wc_s_free(); W2_s_free(); W1_s_free()

    return b


# ---------------------------------------------------------------- entry point

def make_in_maps(inputs, p, n=None, ncores=NC):
    if n is None:
        n = N
    d = p['d']
    NPC, NB128 = d['NPC'], d['NB128']
    NPAD = NB128 * 128
    NBLK = NB128
    x = np.asarray(inputs['x'], np.float32)
    W1 = np.asarray(inputs['W1'], np.float32).astype(BF16NP)
    W2 = np.asarray(inputs['W2'], np.float32).astype(BF16NP)
    Wc = np.asarray(inputs['Wc'], np.float32).astype(BF16NP)
    b1 = np.asarray(inputs['b1'], np.float32)
    b2 = np.asarray(inputs['b2'], np.float32)
    bc = np.asarray(inputs['bc'], np.float32)
    dinv = p['dinv']

    b1c = b1.reshape(HID // 128, 128).T.copy()
    b2c = b2.reshape(MID // 128, 128).T.copy()
    bcr = np.tile(bc[None, :], (128, 1))
    iota = np.tile(np.arange(BLK, dtype=np.float32)[None, :],
                   (128, MCHUNK)).astype(BF16NP)
    id8 = np.eye(128, dtype=np.float32).astype(FP8NP if USE_FP8 else BF16NP)
    id16 = np.eye(128, dtype=np.float32).astype(BF16NP)

    in_maps = []
    for c in range(ncores):
        rows = slice(c * NPC, (c + 1) * NPC)
        xT = np.zeros((IN, NPAD), BF16NP)
        xT[:, :NPC] = x[rows].T.astype(BF16NP)
        dv = dinv[rows]
        tmpv = np.zeros(NB128 * 128, np.float32)
        tmpv[:NPC] = dv
        dvo = tmpv.reshape(NB128, 128).T.copy()      # dinv by (row%128, row//128)
        dvd = np.zeros((128, NBLK * BLK), BF16NP)    # dinv[dst] replicated /parts
        dvd[:, :NPC] = np.tile(dv.astype(BF16NP)[None, :], (128, 1))
        pc = p['per_core'][c]
        in_maps.append(dict(
            xT=xT, W1=W1, W2=W2, Wc=Wc, b1c=b1c, b2c=b2c, bcr=bcr,
            idx_lo=pc['idx_lo'], idx_hi=pc['idx_hi'], dl=pc['dl'],
            iota=iota, id8=id8, id16=id16, dvo=dvo, dvd=dvd))
    return in_maps


def kernel_with_results(inputs, trace=False, **kw):
    from concourse import bacc
    from concourse import bass_utils

    edge_index = np.asarray(inputs['edge_index'])
    p = plan(edge_index, n=N)
    nc_obj = bacc.Bacc("TRN2", target_bir_lowering=False, debug=False,
                       num_devices=NC, num_swdge_queues=4)
    build(nc_obj, p['sched'], p['NT'], p['TL'], p['TH'], n=N)
    nc_obj.compile()
    in_maps = make_in_maps(inputs, p, n=N)
    res = bass_utils.run_bass_kernel_spmd(nc_obj, in_maps,
                                          core_ids=list(range(NC)),
                                          trace=trace, **kw)
    out = np.concatenate([np.asarray(res.results[c]['out']) for c in range(NC)],
                         axis=0)
    return out.astype(np.float32), res


def kernel(**inputs) -> np.ndarray:
    return kernel_with_results(inputs)[0]


if __name__ == '__main__':
    import reference
    inputs = {k: np.asarray(v) for k, v in reference.setup_inputs().items()}
    out = kernel(**inputs)
    print('kernel out', out.shape, out.dtype)
